# revision 8
# baseline (speedup 1.0000x reference)
"""Trainium2 Bass kernel for nn_BertIntermediate (QuantizeLinear + exact GELU).

Reference computation (see harness reference):
    xq = fake_quant(x)   # symmetric per-tensor int8 fake quant, scale = max|x|/127
    Wq = fake_quant(W)
    h  = xq @ Wq.T + b
    out = h * 0.5 * (1 + erf(h/sqrt(2)))

Key numerical insight: q = round(v/scale) is an integer in [-127, 127], exactly
representable in bf16 (8-bit mantissa holds integers up to 256). Products are
<= 127*128 and k-sums over H=1024 stay below 2^24, so a bf16 matmul with fp32
PSUM accumulation reproduces the fp32 reference EXACTLY (up to rounding-tie
flips worth ~1e-3 absolute). The scales factor out of the GEMM:
    h = (sx*sw) * (qx @ qW.T) + b
and fold into the scalar-engine activation (gelu(scale*psum + bias)).

Sharding (8 cores): 2D grid, 4-way over tokens x 2-way over intermediate dim.
Per core: x^T quarter [1024, 2048] replicated x2, W^T half [1024, 2048]
replicated x4, output block written transposed [2048 I, 2048 tok].
This costs ~34 MB DMA per core vs ~53 MB for the pure Megatron column split,
balancing DMA (~100us) against PE (~110us) at the roofline ridge.

The global quantization scales need max|x|, max|W| over the FULL tensors: each
core reduces a distinct 1/8 shard (passed as extra inputs), then a tiny
AllReduce(max) of 2 floats across the 8 cores combines them on-device.

Rounding: round-half-to-even via the fp32 magic-number trick
    rne(v) = (v + 1.5*2^23) - 1.5*2^23        (exact for |v| <= 2^22)
which matches jnp.round's banker's rounding.
"""

import numpy as np

import concourse.bass as bass
import concourse.bass_isa as bass_isa
import concourse.mybir as mybir
from concourse import bass_utils
from concourse.tile import TileContext

F32 = mybir.dt.float32
BF16 = mybir.dt.bfloat16
MAGIC = 12582912.0  # 1.5 * 2**23: fp32 add/sub rounds to nearest int (RNE)
N_CORES = 8
TI, II = 4, 2  # token-quarters x intermediate-halves

# Full problem dims
B, S, H, I = 16, 512, 1024, 4096
M = B * S  # 8192 tokens


def _split_sync_waits(nc, max_waits=1):
    """Walrus in this container rejects instructions carrying more than a
    couple of sync-wait commands ("Too many sync wait commands"). Hoist excess
    waits onto single-wait nops inserted just before the instruction on the
    same engine queue — sequencers process in order, so semantics are
    unchanged."""
    n = 0
    for fn in nc.m.functions:
        for blk in fn.blocks:
            new_insts = []
            for inst in blk.instructions:
                si = inst.sync_info
                waits = list(si.on_wait or []) if si is not None else []
                if len(waits) > max_waits:
                    keep = waits[-max_waits:]
                    for w in waits[:-max_waits]:
                        n += 1
                        nop = mybir.InstNoOp(
                            name=f"I-waitsplit-{n}",
                            ins=[],
                            outs=[],
                            engine=inst.engine,
                        )
                        nop.sync_info = mybir.SyncInfo(on_wait=[w], on_update=[])
                        new_insts.append(nop)
                    inst.sync_info = mybir.SyncInfo(
                        on_wait=keep, on_update=list(si.on_update or [])
                    )
                new_insts.append(inst)
            blk.instructions = new_insts


def build(h=H, m_core=M // TI, i_core=I // II, xsh_cols=None, wsh_cols=None):
    """Build the SPMD Bass program for one core's block.

    h:      contraction dim (multiple of 128)
    m_core: tokens per core (multiple of 512)
    i_core: intermediate outputs per core (multiple of 128)
    """
    if xsh_cols is None:
        xsh_cols = m_core // II
    if wsh_cols is None:
        wsh_cols = i_core // TI
    kt = h // 128          # contraction tiles
    n_it = i_core // 128   # output I-tiles (PSUM partition dim)
    n_tg = m_core // 512   # token groups (PSUM free dim)
    n_bt = (n_it + 15) // 16  # bias tile columns padded into [128, n_it]

    nc = bass.Bass(num_devices=N_CORES)
    xT = nc.dram_tensor("xT", [h, m_core], F32, kind="ExternalInput")
    wT = nc.dram_tensor("wT", [h, i_core], F32, kind="ExternalInput")
    xsh = nc.dram_tensor("xsh", [h, xsh_cols], F32, kind="ExternalInput")
    wsh = nc.dram_tensor("wsh", [h, wsh_cols], F32, kind="ExternalInput")
    bias = nc.dram_tensor("bias", [128, n_it], F32, kind="ExternalInput")
    outT = nc.dram_tensor("outT", [i_core, m_core], F32, kind="ExternalOutput")
    cc_in = nc.dram_tensor("cc_in", [1, 2], F32, kind="Internal")
    cc_out = nc.dram_tensor("cc_out", [1, 2], F32, kind="Internal", addr_space="Shared")
    scr = nc.dram_tensor("scr", [128, 2], F32, kind="Internal")

    groups = [list(range(N_CORES))]

    with TileContext(nc) as tc:
        with (
            tc.tile_pool(name="res", bufs=1) as res,
            tc.tile_pool(name="stage", bufs=3) as stage,
            tc.tile_pool(name="small", bufs=1) as small,
            tc.tile_pool(name="psum", bufs=8, space="PSUM") as pp,
            tc.tile_pool(name="evac", bufs=4) as evac,
        ):
            # ---------- phase 0: local max|shard|, AllReduce, scales ----------
            macc = small.tile([128, 2 * kt], F32, tag="macc")
            for k in range(kt):
                xs = stage.tile([128, xsh_cols], F32, tag="xs")
                nc.sync.dma_start(xs[:], xsh[k * 128:(k + 1) * 128, :])
                nc.vector.tensor_reduce(
                    macc[:, k:k + 1], xs[:], axis=mybir.AxisListType.X,
                    op=mybir.AluOpType.max, apply_absolute_value=True,
                )
            for k in range(kt):
                ws = stage.tile([128, wsh_cols], F32, tag="ws")
                nc.sync.dma_start(ws[:], wsh[k * 128:(k + 1) * 128, :])
                nc.vector.tensor_reduce(
                    macc[:, kt + k:kt + k + 1], ws[:], axis=mybir.AxisListType.X,
                    op=mybir.AluOpType.max, apply_absolute_value=True,
                )
            gm2 = small.tile([128, 2], F32, tag="gm2")
            nc.vector.tensor_reduce(
                gm2[:, 0:1], macc[:, 0:kt], axis=mybir.AxisListType.X,
                op=mybir.AluOpType.max,
            )
            nc.vector.tensor_reduce(
                gm2[:, 1:2], macc[:, kt:2 * kt], axis=mybir.AxisListType.X,
                op=mybir.AluOpType.max,
            )
            # partition-max via DRAM round-trip transposing gather (the custom
            # GPSIMD partition ops fail codegen in this walrus build)
            nc.sync.dma_start(scr[:, :], gm2[:])
            g3 = small.tile([1, 2, 128], F32, tag="g3")
            nc.sync.dma_start(g3[:], bass.AP(scr, 0, [[0, 1], [1, 2], [2, 128]]))
            lmax = small.tile([1, 2], F32, tag="lmax")
            nc.vector.tensor_reduce(
                lmax[:], g3[:], axis=mybir.AxisListType.X, op=mybir.AluOpType.max
            )
            # AllReduce(max) of [max|x|, max|W|] across the 8 cores
            nc.sync.dma_start(cc_in[:, :], lmax[:])
            nc.gpsimd.collective_compute(
                "AllReduce", mybir.AluOpType.max, replica_groups=groups,
                ins=[cc_in[:, :]], outs=[cc_out[:, :]],
            )
            gmx = small.tile([128, 2], F32, tag="gmx")
            nc.sync.dma_start(gmx[:], cc_out[0:1, :].broadcast_to([128, 2]))
            # scales: s = gmax/127 (matches jnp max/127), inv = 1/s, ss = sx*sw
            sxsw = small.tile([128, 2], F32, tag="sxsw")
            nc.vector.tensor_scalar_mul(sxsw[:], gmx[:], 1.0 / 127.0)
            inv = small.tile([128, 2], F32, tag="inv")
            nc.vector.reciprocal(inv[:], sxsw[:])
            ss = small.tile([128, 1], F32, tag="ss")
            nc.vector.tensor_tensor(
                ss[:], sxsw[:, 0:1], sxsw[:, 1:2], op=mybir.AluOpType.mult
            )
            bt = small.tile([128, n_it], F32, tag="bt")
            nc.sync.dma_start(bt[:], bias[:, :])

            # ---------- phase 1: quantize W then x into resident bf16 ----------
            CH = 1024  # quantize chunk width (free dim)
            wq = res.tile([128, kt * i_core], BF16, tag="wq")  # [p, k, I]
            xq = res.tile([128, kt * m_core], BF16, tag="xq")  # [p, k, tok]
            for k in range(kt):
                for c0 in range(0, i_core, CH):
                    cw = min(CH, i_core - c0)
                    wf = stage.tile([128, CH], F32, tag="wf")
                    nc.sync.dma_start(
                        wf[:, :cw], wT[k * 128:(k + 1) * 128, c0:c0 + cw]
                    )
                    t1 = stage.tile([128, CH], F32, tag="t1")
                    nc.scalar.activation(
                        t1[:, :cw], wf[:, :cw],
                        mybir.ActivationFunctionType.Copy,
                        bias=MAGIC, scale=inv[:, 1:2],
                    )
                    nc.vector.tensor_scalar(
                        wq[:, k * i_core + c0:k * i_core + c0 + cw],
                        t1[:, :cw], MAGIC, None, op0=mybir.AluOpType.subtract,
                    )
            for k in range(kt):
                for c0 in range(0, m_core, CH):
                    cw = min(CH, m_core - c0)
                    xf = stage.tile([128, CH], F32, tag="xf")
                    nc.sync.dma_start(
                        xf[:, :cw], xT[k * 128:(k + 1) * 128, c0:c0 + cw]
                    )
                    t2 = stage.tile([128, CH], F32, tag="t2")
                    nc.scalar.activation(
                        t2[:, :cw], xf[:, :cw],
                        mybir.ActivationFunctionType.Copy,
                        bias=MAGIC, scale=inv[:, 0:1],
                    )
                    nc.vector.tensor_scalar(
                        xq[:, k * m_core + c0:k * m_core + c0 + cw],
                        t2[:, :cw], MAGIC, None, op0=mybir.AluOpType.subtract,
                    )

            # ---------- phase 2: integer bf16 matmul + fused gelu ----------
            for i in range(n_it):
                ps = [
                    pp.tile([128, 512], F32, tag="ps", name=f"ps_{i}_{tg}")
                    for tg in range(n_tg)
                ]
                for k in range(kt):
                    lhsT = wq[:, k * i_core + i * 128:k * i_core + (i + 1) * 128]
                    for tg in range(n_tg):
                        rhs = xq[:, k * m_core + tg * 512:k * m_core + (tg + 1) * 512]
                        nc.tensor.matmul(
                            ps[tg][:], lhsT, rhs,
                            start=(k == 0), stop=(k == kt - 1),
                        )
                for tg in range(n_tg):
                    ot = evac.tile([128, 512], F32, tag="ot")
                    nc.scalar.activation(
                        ot[:], ps[tg][:],
                        mybir.ActivationFunctionType.Gelu,
                        bias=bt[:, i:i + 1], scale=ss[:, 0:1],
                    )
                    nc.sync.dma_start(
                        outT[i * 128:(i + 1) * 128, tg * 512:(tg + 1) * 512], ot[:]
                    )
    _split_sync_waits(nc)
    return nc


_CACHE: dict = {}


def _get_nc():
    if "nc" not in _CACHE:
        _CACHE["nc"] = build()
    return _CACHE["nc"]


def shard_inputs(x, W, b):
    """Host-side sharding: pure layout (transpose/slice/replicate), no math."""
    x2 = np.ascontiguousarray(x.reshape(M, H).T)  # [H, M]
    in_maps = []
    for c in range(N_CORES):
        ti, ii = c // II, c % II
        mq, ih = M // TI, I // II
        xT = np.ascontiguousarray(x2[:, ti * mq:(ti + 1) * mq])
        wT = np.ascontiguousarray(W[ii * ih:(ii + 1) * ih, :].T)
        # distinct 1/8 shards for the global max reduction
        xs = np.ascontiguousarray(xT[:, ii * (mq // II):(ii + 1) * (mq // II)])
        wss = np.ascontiguousarray(wT[:, ti * (ih // TI):(ti + 1) * (ih // TI)])
        bia = np.ascontiguousarray(
            b[ii * ih:(ii + 1) * ih].reshape(ih // 128, 128).T
        )
        in_maps.append(
            {"xT": xT, "wT": wT, "xsh": xs, "wsh": wss, "bias": bia}
        )
    return in_maps


def unshard_output(results):
    """Assemble per-core transposed blocks into the full [B, S, I] output."""
    outT = np.empty((I, M), np.float32)
    for c in range(N_CORES):
        ti, ii = c // II, c % II
        mq, ih = M // TI, I // II
        outT[ii * ih:(ii + 1) * ih, ti * mq:(ti + 1) * mq] = results[c]["outT"]
    return np.ascontiguousarray(outT.T).reshape(B, S, I)


def kernel(x, W, b):
    nc = _get_nc()
    in_maps = shard_inputs(
        np.asarray(x, np.float32), np.asarray(W, np.float32), np.asarray(b, np.float32)
    )
    res = bass_utils.run_bass_kernel_spmd(nc, in_maps, core_ids=list(range(N_CORES)))
    return unshard_output(res.results)
